# revision 11
# baseline (speedup 1.0000x reference)
"""Trainium2 Bass kernel for nn_BayesianLayer (sampling, data-parallel over batch).

Reference computation (per full inputs):
    sigma      = softplus(ro)                  # [IN, OUT]
    sigma_b    = softplus(ro_bias)             # [1, OUT]
    weights    = eps * sigma + mu              # [B, IN, OUT]
    bias       = eps_bias * sigma_b + mu_bias  # [B, OUT]
    out        = einsum("bi,bio->bo", x, weights) + bias

Sharding: batch B=64 split across 8 NeuronCores (8 samples/core). eps, x,
eps_bias are sharded along batch; mu/ro/mu_bias/ro_bias are replicated.

Per-core algorithm (BL=8 local samples):
  - ACT computes sigma = softplus(ro) once, resident in SBUF (4 MB).
  - PE computes xmu = x_local @ mu with one M=8 matmul chain (mu streamed).
  - comb8[b,:] = eps_bias[b]*sigma_b + mu_bias + xmu[b] combined on DVE.
  - Streaming loop over (b, chunk): DMA 1 MB eps chunks, DVE multiplies by
    sigma writing a float32r product tile, PE reduces over IN with
    per-sample matvecs (lhsT = x column, M=1) accumulating into PSUM.
    float32r runs the PE at full rate (1 cycle/row vs 4 for fp32); the BIR
    verifier requires every matmul operand to be produced by an
    f32r-rounding instruction, hence the dedicated f32r tiles.
    A final K=8 matmul with lhsT = identity column b adds comb8[b].
  - Epilogue: ACT copies the PSUM row to SBUF, DMA writes it to DRAM.

build_nc(repeat=N) wraps the whole body in a For_i loop — used only by the
timing harness (test.py); the graded path uses repeat=1.
"""

import contextlib
import os

import numpy as np

import concourse.bass as bass
import concourse.mybir as mybir
import concourse.tile as tile
from concourse import bacc
from concourse.bass import ts
from concourse import bass_utils
from concourse.masks import make_identity

B, IN, OUT = 64, 1024, 1024
NCORES = 8
BL = B // NCORES  # local batch per core
P = 128           # SBUF partitions
KT = IN // P      # 8 k-tiles of 128 rows
KC = int(os.environ.get("BAYES_KC", "2"))  # k-tiles per eps chunk
CHUNKS = KT // KC
NF = 512          # matmul moving free dim (one PSUM bank of fp32)
# (k_start, n_k_tiles) chunk schedules: the first sigma/eps chunks are a
# single k-tile so the PE pipeline fills early; then full KC-tile chunks
SCHED_RAMP = [(0, 1), (1, 3), (4, 4)]
SCHED_STEADY = [(c * KC, KC) for c in range(CHUNKS)]
# descending tail for the last sample: its final chunk is one k-tile, so
# the post-last-DMA critical path (DVE product + epilogue) is minimal
SCHED_TAIL = [(0, 4), (4, 3), (7, 1)]

F32 = mybir.dt.float32
F32R = mybir.dt.float32r
AF = mybir.ActivationFunctionType

EPS_BUFS = int(os.environ.get("BAYES_EPS_BUFS", "5"))
USE_F32R = os.environ.get("BAYES_MM_DTYPE", "f32") == "f32r"
MMDT = F32R if USE_F32R else F32
# "native" Softplus has no ACT table set in this toolchain; expln uses the
# natural_log_exp_and_others set (one table load for both exp and ln).
SOFTPLUS = os.environ.get("BAYES_SOFTPLUS", "expln")  # native | expln
# column-tiling: run NGRP slices of each matvec concurrently in PE
# col-groups 0/32/64/96 (separate XBUSes) — divides eps PE time by NGRP
NGRP = int(os.environ.get("BAYES_NGRP", "4"))  # 1 | 2 | 4
COLT = NGRP > 1
NFG = OUT // NGRP          # per-group moving free dim
PTOP = 32 * (NGRP - 1) + 1  # psum/row tile partition extent


def _softplus(nc, out, in_):
    if SOFTPLUS == "native":
        nc.scalar.activation(out, in_, AF.Softplus)
    else:  # ln(exp(x) + 1) — CoreSim-compatible, exp/ln share one table set
        nc.scalar.activation(out, in_, AF.Exp)
        nc.scalar.activation(out, out, AF.Ln, bias=1.0)


def build_nc(repeat: int = 1) -> bass.Bass:
    nc = bacc.Bacc(
        "TRN2",
        target_bir_lowering=False,
        debug=False,
        num_devices=NCORES,
    )

    x_d = nc.dram_tensor("x", [BL, IN], F32, kind="ExternalInput")
    mu_d = nc.dram_tensor("mu", [IN, OUT], F32, kind="ExternalInput")
    ro_d = nc.dram_tensor("ro", [IN, OUT], F32, kind="ExternalInput")
    mub_d = nc.dram_tensor("mu_bias", [1, OUT], F32, kind="ExternalInput")
    rob_d = nc.dram_tensor("ro_bias", [1, OUT], F32, kind="ExternalInput")
    eps_d = nc.dram_tensor("eps", [BL, IN, OUT], F32, kind="ExternalInput")
    ebd_d = nc.dram_tensor("eps_bias", [BL, OUT], F32, kind="ExternalInput")
    out_d = nc.dram_tensor("out", [BL, OUT], F32, kind="ExternalOutput")

    with tile.TileContext(nc) as tc:
        with (
            tc.tile_pool(name="const", bufs=1) as const,
            tc.tile_pool(name="stream", bufs=EPS_BUFS) as stream,
            tc.tile_pool(name="rows", bufs=3) as rows,
            tc.tile_pool(name="psum_acc", bufs=4, space="PSUM") as psum_acc,
            tc.tile_pool(name="psum_misc", bufs=2, space="PSUM") as psum_misc,
        ):
          # input-independent constant, hoisted out of the timing loop
          ident = const.tile([BL, BL], F32, name="ident")
          make_identity(nc, ident)
          with tc.For_i(0, repeat, 1) if repeat > 1 else contextlib.nullcontext():
            # ---------- setup ----------
            # xT[p, k, b] = x[b, k*128 + p] via regular identity-rhs matmuls:
            # pt = x_slice.T @ I8 (transpose-mode matmul crashes the device
            # in this toolchain; a plain matmul with identity rhs is exact)
            x_sb = const.tile([BL, IN], F32, name="x_sb")
            nc.scalar.dma_start(x_sb, x_d[:])
            if USE_F32R:
                x_sbr = const.tile([BL, IN], F32R, name="x_sbr")
                nc.vector.tensor_copy(x_sbr, x_sb)
                ident_r = const.tile([BL, BL], F32R, name="ident_r")
                nc.vector.tensor_copy(ident_r, ident)
            else:
                x_sbr, ident_r = x_sb, ident
            xT_r = const.tile([P, KT, BL], MMDT, name="xT_r")
            for k in range(KT):
                pt = psum_misc.tile([P, BL], F32, name="pt", tag="xmu")
                nc.tensor.matmul(
                    pt, x_sbr[:, ts(k, P)], ident_r, start=True, stop=True
                )
                nc.vector.tensor_copy(xT_r[:, k, :], pt)

            # sigma = softplus(ro), resident [128, 8, 1024]
            sig = const.tile([P, KT, OUT], F32, name="sig")
            ro_k = ro_d[:].rearrange("(k p) o -> p k o", p=P)
            for ks, cnt in SCHED_RAMP:
                rot = stream.tile([P, cnt, OUT], F32, name="rot", tag="bigtile")
                nc.sync.dma_start(rot, ro_k[:, ks : ks + cnt, :])
                _softplus(nc, sig[:, ks : ks + cnt, :], rot)

            # xmu/bias block, emitted between b=0 and b=1 so the PE can
            # start on eps as soon as sigma chunk 0 lands (mu loads later,
            # off the critical path)
            if COLT:
                comb32 = const.tile([PTOP, BL, NFG], F32, name="comb32")
            else:
                comb1 = const.tile([1, BL, OUT], F32, name="comb1")

            def emit_mu_and_bias():
                # bias inputs first: small DMAs land early, ACT/DVE compute
                # overlaps the mu stream below
                rb8 = const.tile([BL, OUT], F32, name="rb8")
                mb8 = const.tile([BL, OUT], F32, name="mb8")
                nc.scalar.dma_start(rb8, rob_d[:].to_broadcast((BL, OUT)))
                nc.scalar.dma_start(mb8, mub_d[:].to_broadcast((BL, OUT)))
                sb8 = const.tile([BL, OUT], F32, name="sb8")
                _softplus(nc, sb8, rb8)
                eb8 = const.tile([BL, OUT], F32, name="eb8")
                nc.scalar.dma_start(eb8, ebd_d[:])
                xmu_ps = psum_misc.tile([BL, OUT], F32, name="xmu_ps", tag="xmu")
                mu_r = mu_d[:].rearrange("(c j p) o -> c p j o", p=P, j=KC)
                for c in range(CHUNKS):
                    mut = stream.tile([P, KC, OUT], F32, name="mut", tag="bigtile")
                    nc.sync.dma_start(mut, mu_r[c])
                    if USE_F32R:
                        mut_r = stream.tile(
                            [P, KC, OUT], F32R, name="mut_r", tag="bigtile_r"
                        )
                        nc.vector.tensor_copy(mut_r, mut)
                    else:
                        mut_r = mut
                    for j in range(KC):
                        k = c * KC + j
                        for h in range(2):
                            nc.tensor.matmul(
                                xmu_ps[:, ts(h, NF)],
                                xT_r[:, k, :],
                                mut_r[:, j, ts(h, NF)],
                                start=(k == 0),
                                stop=(k == KT - 1),
                            )
                comb8 = const.tile([BL, OUT], F32, name="comb8")
                nc.vector.tensor_mul(comb8, eb8, sb8)
                nc.vector.tensor_add(comb8, comb8, mb8)
                nc.vector.tensor_add(comb8, comb8, xmu_ps)
                # partition-0/32 reshape: per-sample epilogue adds are
                # partition-aligned (DVE has no cross-lane path)
                if COLT:
                    for g in range(NGRP):
                        nc.scalar.dma_start(
                            comb32[32 * g : 32 * g + 1, :, :],
                            comb8[:, ts(g, NFG)],
                        )
                else:
                    nc.scalar.dma_start(comb1, comb8)

            # ---------- streaming main loop ----------
            # epilogues are deferred one iteration: comb1 (written by the
            # mu/bias block emitted at b==1) must exist before any row add
            eps_k = eps_d[:].rearrange("b (k p) o -> b p k o", p=P)

            # outputs stage in SBUF ([g*32, b, :] = out[b, g*256:(g+1)*256]);
            # one DMA ships all 8 rows at iteration end
            if COLT:
                stage = const.tile([P, BL, NFG], F32, name="stage")

            def emit_epilogue(b, ps):
                # NB: a fused DVE tensor_add(row, ps(PSUM), comb(SBUF)) is
                # fatal on HW (NRT_EXEC_UNIT_UNRECOVERABLE) — evacuate via
                # ACT first
                if COLT:
                    # one ACT copy + one DVE add over all 4 col-groups at
                    # once (cost scales with free dim, not partitions; the
                    # untouched partitions carry garbage that is never read)
                    nc.scalar.copy(stage[0:PTOP, b, :], ps)
                    nc.vector.tensor_add(
                        stage[0:PTOP, b, :], stage[0:PTOP, b, :], comb32[:, b, :]
                    )
                else:
                    row = rows.tile([1, OUT], F32, name="row", tag="row")
                    nc.scalar.copy(row, ps)
                    nc.vector.tensor_add(row, row, comb1[0:1, b, :])
                    nc.scalar.dma_start(out_d[b : b + 1, :], row)

            def emit_out_dma():
                if COLT:
                    stage_v = stage[:].rearrange(
                        "(g r) b n -> g r b n", r=32
                    )[:, 0, :, :]
                    nc.scalar.dma_start(
                        out_d[:].rearrange("b (g n) -> g b n", g=NGRP), stage_v
                    )

            emit_mu_and_bias()
            pending = []
            for b in range(BL):
                if COLT:
                    # group g lives at partition 32g of one PSUM bank
                    ps = psum_acc.tile([PTOP, NFG], F32, name="ps", tag="ps")
                else:
                    ps = psum_acc.tile([1, OUT], F32, name="ps", tag="ps")
                if b == 0:
                    sched = SCHED_RAMP
                elif b == BL - 1:
                    sched = SCHED_TAIL
                else:
                    sched = SCHED_STEADY
                for ks, cnt in sched:
                    ep = stream.tile([P, cnt, OUT], F32, name="ep", tag="bigtile")
                    nc.sync.dma_start(ep, eps_k[b][:, ks : ks + cnt, :])
                    if USE_F32R:
                        epr = stream.tile(
                            [P, cnt, OUT], F32R, name="epr", tag="bigtile_r"
                        )
                    else:
                        epr = ep
                    nc.vector.tensor_tensor(
                        epr, ep, sig[:, ks : ks + cnt, :], mybir.AluOpType.mult
                    )
                    for j in range(cnt):
                        k = ks + j
                        if COLT:
                            for g in range(NGRP):
                                nc.tensor.matmul(
                                    ps[32 * g : 32 * g + 1, :],
                                    xT_r[:, k, b : b + 1],
                                    epr[:, j, ts(g, NFG)],
                                    start=(k == 0),
                                    stop=(k == KT - 1),
                                    tile_position=(0, 32 * g),
                                )
                        else:
                            for h in range(2):
                                nc.tensor.matmul(
                                    ps[:, ts(h, NF)],
                                    xT_r[:, k, b : b + 1],
                                    epr[:, j, ts(h, NF)],
                                    start=(k == 0),
                                    stop=(k == KT - 1),
                                )
                pending.append((b, ps))
                if b >= 1:
                    emit_epilogue(*pending[b - 1])
            emit_epilogue(*pending[BL - 1])
            emit_out_dma()

    nc.finalize()
    return nc


def _shard_inputs(inputs: dict) -> list[dict]:
    x = np.ascontiguousarray(np.asarray(inputs["x"], dtype=np.float32))
    mu = np.ascontiguousarray(np.asarray(inputs["mu"], dtype=np.float32))
    ro = np.ascontiguousarray(np.asarray(inputs["ro"], dtype=np.float32))
    mub = np.ascontiguousarray(np.asarray(inputs["mu_bias"], dtype=np.float32))
    rob = np.ascontiguousarray(np.asarray(inputs["ro_bias"], dtype=np.float32))
    eps = np.ascontiguousarray(np.asarray(inputs["eps"], dtype=np.float32))
    ebd = np.ascontiguousarray(np.asarray(inputs["eps_bias"], dtype=np.float32))

    in_maps = []
    for k in range(NCORES):
        sl = slice(k * BL, (k + 1) * BL)
        in_maps.append(
            {
                "x": np.ascontiguousarray(x[sl]),
                "mu": mu,
                "ro": ro,
                "mu_bias": mub,
                "ro_bias": rob,
                "eps": np.ascontiguousarray(eps[sl]),
                "eps_bias": np.ascontiguousarray(ebd[sl]),
            }
        )
    return in_maps


def run(inputs: dict, trace: bool = False):
    nc = build_nc()
    in_maps = _shard_inputs(inputs)
    res = bass_utils.run_bass_kernel_spmd(
        nc, in_maps, core_ids=list(range(NCORES)), trace=trace
    )
    out = np.concatenate([res.results[k]["out"] for k in range(NCORES)], axis=0)
    return out.astype(np.float32), res


def kernel(**inputs: np.ndarray) -> np.ndarray:
    try:
        out, _ = run(inputs, trace=False)
    except Exception:
        # transient device errors (NRT_EXEC_UNIT_UNRECOVERABLE) have been
        # observed to clear on retry
        import time

        time.sleep(5.0)
        out, _ = run(inputs, trace=False)
    return out

